# revision 46
# baseline (speedup 1.0000x reference)
"""Llama SDPA attention (GQA + RoPE + causal) on 8 Trainium2 NeuronCores.

Sharding: DP-2 over batch x TP-4 over heads. Core c = 4*b + g handles
batch b and head group g (8 q heads, 2 kv heads). Each core computes its
partial o_proj output [T, C] (Wo split along the input-feature dim); the
partials are summed on the host (the gather/unshard step).

v2 kernel notes (on top of the v1 design):
  - Phase A in bf16 (x moving + W stationary, NE=4 chunks of 512 tokens):
    same PE rate as fp32r, half the DMA bytes, half the RoPE/copy
    instruction count. Wv is host-scaled x16 so v is fp8-friendly; the
    softmax ones-column is 16 so normalization cancels the scale exactly.
  - RoPE: the straight mul reads the projection PSUM directly (engine ops
    may read PSUM/SBUF operands at shifted partition bases); only the
    +-32 partition-swapped operand needs Act copies. The rotated q is
    scatter-added straight into the packed q layout.
  - Attention processes key strips in PAIRS: QK writes a [128,2,512] PSUM
    pair tile, one Act exp converts both strips to fp8e4, and one
    DoubleRow fp8 matmul (2 strips = 256-deep contraction) accumulates PV.
    The diagonal strip stays fp32r end-to-end (exp -> f32r p, f32r PV via
    a separate f32r v copy) - early-token rows see no fp8 error, and
    off-diagonal fp8 rounding averages out over many keys (~6e-3 total).
    The causal mask is seeded into the diag PSUM by an identity matmul so
    no DVE hop sits on the Act critical path. Strips run in interleaved
    big/small order so small strips' normalize/o_proj latency hides under
    big strips' attention.
  - o_proj stays fp32r (fp8 would need both-operand residual compensation
    to pass the accuracy gate - not worth it), is software-pipelined by
    one strip, and its PSUM rides the po accumulator rings (8-bank fit).
  - DMA count minimized (~23/rep): single-shot weights, packed rope
    tables, hoisted v-template constants, and one output DMA per strip
    pair (15-k, k) via a strided DRAM access pattern. Output is bf16.
"""

import sys
import numpy as np

if '/opt/trn_rl_repo' not in sys.path:
    sys.path.insert(0, '/opt/trn_rl_repo')

B, T, C = 2, 2048, 2048
NH, NKV, HD = 32, 8, 64
G = 4              # head groups (TP degree)
QF = NH // G * HD  # 512 q features per core
KF = NKV // G * HD # 128 k/v features per core
NE = 4             # token chunks in projection phase
TE = T // NE       # 256
NCC = C // 128     # 16 contraction chunks
NG = 4             # weight ci-groups (startup overlap)
CG = NCC // NG
NTS = T // 128     # 16 token strips (attention)
VW = 80            # padded v free width for DoubleRow (65 -> 80, 16B align)
VSCALE = 16.0      # host scale on Wv; ones column = VSCALE cancels it

_CACHE = {}


def _rope_perm(nheads):
    """Per-head feature permutation: [d0,d2,...,d62, d1,d3,...,d63]."""
    p = []
    for h in range(nheads):
        base = h * HD
        p.extend(base + d for d in range(0, HD, 2))
        p.extend(base + d for d in range(1, HD, 2))
    return np.array(p, dtype=np.int64)


def _build_program(causal, rep=1):
    assert causal, "v2 kernel is causal-only (harness mask is causal)"
    import concourse.bass as bass
    import concourse.tile as tile
    import concourse.mybir as mybir
    from concourse import bacc
    from concourse.bass import ts

    F32 = mybir.dt.float32
    F32R = mybir.dt.float32r
    BF16 = mybir.dt.bfloat16
    F8 = mybir.dt.float8e4
    Exp = mybir.ActivationFunctionType.Exp
    DR = mybir.MatmulPerfMode.DoubleRow

    nc = bacc.Bacc("TRN2", target_bir_lowering=False, debug=False)

    # pre-tiled inputs: per-partition data is contiguous in DRAM
    xTd = nc.dram_tensor("xTd", [128, NE, NCC, TE], BF16, kind="ExternalInput").ap()
    wqd = nc.dram_tensor("wqd", [128, NCC, QF], BF16, kind="ExternalInput").ap()
    wkd = nc.dram_tensor("wkd", [128, NCC, KF], BF16, kind="ExternalInput").ap()
    wvd = nc.dram_tensor("wvd", [128, NCC, KF], BF16, kind="ExternalInput").ap()
    wod = nc.dram_tensor("wod", [128, QF // 128, C], F32R, kind="ExternalInput").ap()
    csi = nc.dram_tensor("csi", [128, 2 * T + 64], F32, kind="ExternalInput").ap()
    maskd = nc.dram_tensor("maskd", [128, 512], F32R, kind="ExternalInput").ap()
    id128d = nc.dram_tensor("id128d", [128, 128], F32R, kind="ExternalInput").ap()
    seld = nc.dram_tensor("seld", [33, 128], F32R, kind="ExternalInput").ap()
    v8td = nc.dram_tensor("v8td", [128, 2 * 2 * (NTS // 2) * VW], F8,
                          kind="ExternalInput").ap()
    on16d = nc.dram_tensor("on16d", [128, 2, NTS], F32R,
                           kind="ExternalInput").ap()
    zrecd = nc.dram_tensor("zrecd", [33, 512], F32R,
                           kind="ExternalInput").ap()
    part = nc.dram_tensor("part", [T, C], BF16, kind="ExternalOutput").ap()

    with tile.TileContext(nc) as tc:
        from contextlib import ExitStack
        with ExitStack() as ctx:
            persist = ctx.enter_context(tc.tile_pool(name="persist", bufs=1))
            kT_sbp = persist.tile([128, T], F32R)    # roped k^T (kv0 | kv1)
            qp_sb = persist.tile([128, 4, T], F32R)  # packed q^T [64*kv+d, slot, t]
            SLOT = {0: 0, 1: 2, 2: 1, 3: 3}
            # fp8 v pairs: [p, pair-slot, kv, pairidx, VW]; col 64 = VSCALE,
            # cols 65.. = 0
            v8_sb = persist.tile([128, 2, 2, NTS // 2, VW], F8)
            # f32r v for the diagonal strip: [p, kv, strip, 65]
            v1f_sb = persist.tile([128, 2, NTS, HD + 1], F32R)
            maskd_sb = persist.tile([128, 512], F32R)
            nc.sync.dma_start(maskd_sb, maskd)
            id128_sb = persist.tile([128, 128], F32R)
            nc.sync.dma_start(id128_sb, id128d)
            sel_sb = persist.tile([33, 128], F32R)
            nc.sync.dma_start(sel_sb, seld)
            rec_p = persist.tile([33, 512], F32R)
            nc.sync.dma_start(rec_p, zrecd)
            # constant columns of the v tiles: loaded once (compute never
            # writes them)
            nc.sync.dma_start(v1f_sb[:, :, :, HD:HD + 1].squeeze(), on16d)
            nc.sync.dma_start(
                v8_sb.rearrange("p a b c d -> p (a b c d)"), v8td)

            wo_sb = persist.tile([128, QF // 128, C], F32R)
            for _rep in range(rep):
                # ------------- Phase A: QKV projections + RoPE -------------
                with tc.tile_pool(name="stage_a", bufs=1) as stage_a:
                    vT_sb = stage_a.tile([128, T], F32)
                    csi_sb = stage_a.tile([128, 2 * T + 64], F32)
                    c2_sb = csi_sb[:, 0:T]
                    s2_sb = csi_sb[:, T:2 * T]
                    ident = csi_sb[:, 2 * T:2 * T + 64]

                    with tc.tile_pool(name="weights", bufs=1) as wpool, \
                         tc.tile_pool(name="xpool", bufs=2) as xpool, \
                         tc.tile_pool(name="rtmp", bufs=2) as rpool, \
                         tc.tile_pool(name="psT", bufs=2, space="PSUM") as psT, \
                         tc.tile_pool(name="psA", bufs=4, space="PSUM") as psA:
                        # DMA order: weights first (phase-A critical), then
                        # the first x chunk; rope tables + wo follow.
                        wq_sb = wpool.tile([128, NCC, QF], BF16, name="wq_sb")
                        nc.sync.dma_start(wq_sb, wqd)
                        wk_sb = wpool.tile([128, NCC, KF], BF16, name="wk_sb")
                        nc.sync.dma_start(wk_sb, wkd)
                        wv_sb = wpool.tile([128, NCC, KF], BF16, name="wv_sb")
                        nc.sync.dma_start(wv_sb, wvd)

                        for e in range(NE):
                            x_t = xpool.tile([128, NCC, TE], BF16, tag="x")
                            nc.sync.dma_start(x_t, xTd[:, e, :, :])
                            if e == 0:
                                nc.sync.dma_start(csi_sb, csi)
                            elif e == 1:
                                # phase-B inputs: overlap with remaining A
                                nc.sync.dma_start(wo_sb, wod)
                            for f in range(6):  # 0..3 q-tiles, 4 k, 5 v
                                ps = psA.tile([128, TE], F32, tag="psA")
                                for ci in range(NCC):
                                    if f < 4:
                                        w_ap = wq_sb[:, ci, ts(f, 128)]
                                    elif f == 4:
                                        w_ap = wk_sb[:, ci, :]
                                    else:
                                        w_ap = wv_sb[:, ci, :]
                                    nc.tensor.matmul(
                                        ps, w_ap, x_t[:, ci, :],
                                        start=(ci == 0), stop=(ci == NCC - 1))
                                if f == 5:
                                    nc.scalar.activation(
                                        vT_sb[:, ts(e, TE)], ps,
                                        mybir.ActivationFunctionType.Copy)
                                    # transpose this chunk's two strips now
                                    for j in range(4 * e, 4 * e + 4):
                                        for kvh in range(2):
                                            pst = psT.tile([128, HD], F32,
                                                           tag="vtr")
                                            nc.tensor.transpose(
                                                pst,
                                                vT_sb[64 * kvh:64 * kvh + 64,
                                                      ts(j, 128)],
                                                ident[64 * kvh:64 * kvh + 64,
                                                      :])
                                            nc.scalar.activation(
                                                v1f_sb[:, kvh, j, 0:HD], pst,
                                                mybir.ActivationFunctionType
                                                .Copy)
                                            nc.vector.tensor_copy(
                                                v8_sb[:, j % 2, kvh, j // 2,
                                                      0:HD], pst)
                                    continue
                                # RoPE: Act builds the +-32 partition-swapped
                                # copy; DVE muls read PSUM directly and the
                                # final adds scatter to the packed layouts.
                                shifted = rpool.tile([128, TE], F32, tag="sh")
                                for blk in range(4):
                                    o = 32 * blk
                                    so = o + 32 if blk % 2 == 0 else o - 32
                                    nc.scalar.activation(
                                        shifted[o:o + 32, :],
                                        ps[so:so + 32, :],
                                        mybir.ActivationFunctionType.Copy)
                                t1 = rpool.tile([128, TE], F32, tag="t1")
                                nc.vector.tensor_mul(
                                    t1, ps, c2_sb[:, ts(e, TE)])
                                t2 = rpool.tile([128, TE], F32, tag="t2")
                                nc.vector.tensor_mul(
                                    t2, shifted, s2_sb[:, ts(e, TE)])
                                if f == 4:
                                    nc.vector.tensor_add(
                                        kT_sbp[:, ts(e, TE)], t1, t2)
                                else:
                                    for s in range(2):
                                        h = 2 * f + s
                                        kv, slot = h // 4, SLOT[h % 4]
                                        nc.vector.tensor_add(
                                            qp_sb[64 * kv:64 * kv + 64, slot,
                                                  ts(e, TE)],
                                            t1[64 * s:64 * s + 64, :],
                                            t2[64 * s:64 * s + 64, :])

                # ------------- Phase B: attention + o_proj -------------
                with tc.tile_pool(name="pp", bufs=3) as ppool, \
                     tc.tile_pool(name="norm", bufs=4) as npool, \
                     tc.tile_pool(name="atts", bufs=2) as apool, \
                     tc.tile_pool(name="outs", bufs=2) as opool, \
                     tc.tile_pool(name="psS", bufs=2, space="PSUM") as psS, \
                     tc.tile_pool(name="psO", bufs=2, space="PSUM") as psO:

                    prev = None  # deferred o_proj: (att_s, cstrip, o2, sl)
                    part16 = part.rearrange("(a b) c -> a b c", a=NTS)

                    def emit_oproj_half(att_prev, cs_prev, o2t, sl, ecp):
                        # pc rides the po tag rings: po is double-buffered and
                        # o_proj PSUM reuses the same 4 banks.
                        for sub in range(2):
                            ec = 2 * ecp + sub
                            pc = psO.tile([128, 512], F32, tag=f"po{sub}",
                                          name=f"pc{sub}")
                            for ff in range(4):
                                nc.tensor.matmul(
                                    pc, att_prev[:, ff, :],
                                    wo_sb[:, ff, ts(ec, 512)],
                                    start=(ff == 0), stop=(ff == 3))
                            nc.vector.tensor_copy(
                                o2t[:, sl, ts(ec, 512)], pc)
                        if ecp == 1 and cs_prev < NTS // 2:
                            # pair (NTS-1-k, k) complete -> one DMA for both
                            k = cs_prev
                            dst = part16[k:NTS - k:NTS - 1 - 2 * k, :, :]
                            nc.sync.dma_start(
                                dst.rearrange("a b c -> b a c"), o2t)


                    # interleave big/small strips so the small strips'
                    # normalize + o_proj latency hides under big strips'
                    # attention work
                    order = []
                    for i in range(NTS // 2):
                        order.extend([NTS - 1 - i, i])
                    for cstrip in order:
                        nstr = cstrip + 1
                        po = {}
                        for kv in range(2):
                            po[kv] = psO.tile([128, 512], F32, tag=f"po{kv}",
                                              name=f"po{kv}")[0:VW, :]
                            npairs = (nstr + 1) // 2
                            for pr in range(npairs):
                                j0, j1 = 2 * pr, 2 * pr + 1
                                pss = psS.tile([128, 2, 512], F32, tag="pss")
                                for sl, j in ((0, j0), (1, j1)):
                                    if j > cstrip:
                                        continue
                                    diag = (j == cstrip)
                                    if diag:
                                        # seed the slot with the causal mask
                                        nc.tensor.matmul(
                                            pss[:, sl, :], id128_sb,
                                            maskd_sb, start=True, stop=False,
                                            skip_group_check=True)
                                    nc.tensor.matmul(
                                        pss[:, sl, :],
                                        kT_sbp[64 * kv:64 * kv + 64,
                                               ts(j, 128)],
                                        qp_sb[64 * kv:64 * kv + 64, :,
                                              ts(cstrip, 128)],
                                        start=not diag, stop=True,
                                        skip_group_check=True)
                                dslot = (0 if j0 == cstrip else
                                         (1 if j1 == cstrip else None))
                                if dslot is not None:
                                    # off-diag slot (if any) -> fp8 PV
                                    # (plain fp8 matmul: slot 1 is the diag,
                                    # so no pair partner - skips a memset)
                                    if dslot == 1:
                                        p8 = ppool.tile([128, 2, 512], F8,
                                                        tag="p8")
                                        nc.scalar.activation(
                                            p8[:, 0, :], pss[:, 0, :], Exp,
                                            scale=0.125)
                                        nc.tensor.matmul(
                                            po[kv],
                                            v8_sb[:, 0:1, kv, pr, :]
                                            .squeeze(),
                                            p8[:, 0, :],
                                            start=(pr == 0), stop=False,
                                            skip_group_check=True)
                                    # diagonal strip: f32r p and f32r PV
                                    p32 = ppool.tile([128, 512], F32R,
                                                     tag="p32")
                                    nc.scalar.activation(
                                        p32, pss[:, dslot, :], Exp,
                                        scale=0.125)
                                    nc.tensor.matmul(
                                        po[kv][0:HD + 1, :],
                                        v1f_sb[:, kv, cstrip, :],
                                        p32,
                                        start=(pr == 0 and dslot == 0),
                                        stop=True, skip_group_check=True)
                                else:
                                    p8 = ppool.tile([128, 2, 512], F8,
                                                    tag="p8")
                                    nc.scalar.activation(
                                        p8, pss, Exp, scale=0.125)
                                    nc.tensor.matmul(
                                        po[kv], v8_sb[:, :, kv, pr, :],
                                        p8, start=(pr == 0), stop=False,
                                        perf_mode=DR, skip_group_check=True)
                            if prev is not None:
                                emit_oproj_half(prev[0], prev[1], prev[2], prev[3], kv)

                        # normalize both kv heads: one fused broadcast
                        # (kv1 sums at partition 32: bases must be quadrant-
                        # aligned; rec rows 1..31 are zeroed once at start)
                        nc.vector.tensor_copy(rec_p[0:1, :],
                                              po[0][HD:HD + 1, :])
                        nc.vector.tensor_copy(rec_p[32:33, :],
                                              po[1][HD:HD + 1, :])
                        recb_t = psS.tile([128, 2, 512], F32, tag="pss")
                        nc.tensor.matmul(recb_t[:, 0, :], sel_sb, rec_p,
                                         start=True, stop=True)
                        recb = npool.tile([128, 512], F32, tag="recbs")
                        nc.vector.reciprocal(recb, recb_t[:, 0, :])
                        att_s = apool.tile([128, 4, 128], F32R, tag="att")
                        for kv in range(2):
                            for half in range(2):
                                nc.vector.tensor_mul(
                                    att_s[64 * half:64 * half + 64,
                                          2 * kv:2 * kv + 2, :],
                                    po[kv][0:64, ts(half, 256)].rearrange(
                                        "p (a b) -> p a b", a=2),
                                    recb[64 * kv:64 * kv + 64,
                                         ts(half, 256)].rearrange(
                                        "p (a b) -> p a b", a=2))

                        if cstrip >= NTS // 2:
                            o2t = opool.tile([128, 2, C], BF16, tag="o2",
                                             name="o2t")
                            cur_o2 = o2t
                        sl = 1 if cstrip >= NTS // 2 else 0
                        prev = (att_s, cstrip, cur_o2, sl)
                    emit_oproj_half(prev[0], prev[1], prev[2], prev[3], 0)
                    emit_oproj_half(prev[0], prev[1], prev[2], prev[3], 1)

    nc.finalize()
    return nc


def _host_inputs(x, freqs_cos, freqs_sin, attention_mask, Wq, Wk, Wv, Wo,
                 causal):
    """Build the 8 per-core input maps (pre-tiled for contiguous DMA)."""
    import ml_dtypes
    bf16 = ml_dtypes.bfloat16
    fc = np.asarray(freqs_cos, np.float32)
    fs = np.asarray(freqs_sin, np.float32)
    mask = np.asarray(attention_mask, np.float32)

    # rope factor tables [128, T]; pattern repeats every 64 partitions
    m_idx = np.tile(np.arange(32), 4)                     # p % 32
    c2 = np.ascontiguousarray(fc.T[m_idx])                # [128, T]
    sgn = np.where((np.arange(128) % 64) < 32, -1.0, 1.0).astype(np.float32)
    s2 = np.ascontiguousarray(fs.T[m_idx] * sgn[:, None])
    ident64 = np.ascontiguousarray(
        np.tile(np.eye(64, dtype=np.float32), (2, 1)))
    csi = np.ascontiguousarray(
        np.concatenate([c2, s2, ident64], axis=1)).astype(np.float32)

    # diagonal-block mask, transposed + pre-scaled by 8 (kernel applies *0.125)
    md = (mask[0:128, 0:128].T * 8.0).astype(np.float32)
    maskd = np.ascontiguousarray(
        np.broadcast_to(md[:, None, :], (128, 4, 128))
        .reshape(128, 512)).astype(np.float32)
    id128 = np.eye(128, dtype=np.float32)

    qperm = _rope_perm(NH // G)
    kperm = _rope_perm(NKV // G)

    # recb selector: kv0 sums live at partition 0, kv1 sums at partition 32
    seld = np.zeros((33, 128), np.float32)
    seld[0, 0:64] = 1.0
    seld[32, 64:128] = 1.0

    # v8 template: zeros with the ones-column = VSCALE at col 64
    f8 = ml_dtypes.float8_e4m3
    v8t = np.zeros((128, 2, 2, NTS // 2, VW), np.float32)
    v8t[:, :, :, :, HD] = VSCALE
    v8t = np.ascontiguousarray(v8t.reshape(128, -1)).astype(f8)
    on16 = np.full((128, 2, NTS), VSCALE, np.float32)

    def tile_w(wT, F):
        # [C, F] -> [128, NCC, F], contiguous per partition
        return np.ascontiguousarray(
            np.ascontiguousarray(wT).reshape(NCC, 128, F)
            .transpose(1, 0, 2)).astype(bf16)

    in_maps = []
    for c in range(8):
        b, g = c // 4, c % 4
        xT = np.asarray(x, np.float32)[b].T  # [C, T]
        xtile = np.ascontiguousarray(
            xT.reshape(NCC, 128, NE, TE).transpose(1, 2, 0, 3)).astype(bf16)
        wqT = np.asarray(Wq, np.float32)[g * QF:(g + 1) * QF][qperm].T
        wkT = np.asarray(Wk, np.float32)[g * KF:(g + 1) * KF][kperm].T
        wvT = np.asarray(Wv, np.float32)[g * KF:(g + 1) * KF].T * VSCALE
        woT = np.asarray(Wo, np.float32)[:, g * QF:(g + 1) * QF].T  # [QF, C]
        m = {
            "xTd": xtile,
            "wqd": tile_w(wqT, QF),
            "wkd": tile_w(wkT, KF),
            "wvd": tile_w(wvT, KF),
            "wod": np.ascontiguousarray(
                woT.reshape(QF // 128, 128, C).transpose(1, 0, 2)),
            "csi": csi,
            "maskd": maskd,
            "id128d": id128,
            "seld": seld,
            "v8td": v8t,
            "on16d": on16,
            "zrecd": np.zeros((33, 512), np.float32),
        }
        in_maps.append(m)
    return in_maps


def _detect_causal(mask):
    mask = np.asarray(mask)
    neg = mask.min()
    if neg >= -1e7:
        return False
    tril = np.tril(np.ones((T, T), dtype=bool))
    expect = np.where(tril, np.float32(0.0), np.float32(neg))
    return bool(np.array_equal(mask, expect))


def run(inputs, trace=False):
    from concourse import bass_utils

    causal = _detect_causal(inputs["attention_mask"])
    key = ("prog", causal)
    if key not in _CACHE:
        _CACHE[key] = _build_program(causal)
    nc = _CACHE[key]

    in_maps = _host_inputs(causal=causal, **inputs)
    res = bass_utils.run_bass_kernel_spmd(
        nc, in_maps, core_ids=list(range(8)), trace=trace)

    out = np.empty((B, T, C), np.float32)
    for b in range(B):
        acc = res.results[4 * b]["part"].astype(np.float32)
        for g in range(1, 4):
            acc = acc + res.results[4 * b + g]["part"].astype(np.float32)
        out[b] = acc
    return out, res


def kernel(**inputs):
    out, _ = run(inputs, trace=False)
    return out


# revision 47
# speedup vs baseline: 1.9636x; 1.9636x over previous
"""Llama SDPA attention (GQA + RoPE + causal) on 8 Trainium2 NeuronCores.

Sharding: DP-2 over batch x TP-4 over heads. Core c = 4*b + g handles
batch b and head group g (8 q heads, 2 kv heads). Each core computes its
partial o_proj output [T, C] (Wo split along the input-feature dim); the
partials are summed on the host (the gather/unshard step).

v2 kernel notes (on top of the v1 design):
  - Phase A in bf16 (x moving + W stationary, NE=4 chunks of 512 tokens):
    same PE rate as fp32r, half the DMA bytes, half the RoPE/copy
    instruction count. Wv is host-scaled x16 so v is fp8-friendly; the
    softmax ones-column is 16 so normalization cancels the scale exactly.
  - RoPE: the straight mul reads the projection PSUM directly (engine ops
    may read PSUM/SBUF operands at shifted partition bases); only the
    +-32 partition-swapped operand needs Act copies. The rotated q is
    scatter-added straight into the packed q layout.
  - Attention processes key strips in PAIRS: QK writes a [128,2,512] PSUM
    pair tile, one Act exp converts both strips to fp8e4, and one
    DoubleRow fp8 matmul (2 strips = 256-deep contraction) accumulates PV.
    The diagonal strip stays fp32r end-to-end (exp -> f32r p, f32r PV via
    a separate f32r v copy) - early-token rows see no fp8 error, and
    off-diagonal fp8 rounding averages out over many keys (~6e-3 total).
    The causal mask is seeded into the diag PSUM by an identity matmul so
    no DVE hop sits on the Act critical path. Strips run in interleaved
    big/small order so small strips' normalize/o_proj latency hides under
    big strips' attention.
  - o_proj stays fp32r (fp8 would need both-operand residual compensation
    to pass the accuracy gate - not worth it), is software-pipelined by
    one strip, and its PSUM rides the po accumulator rings (8-bank fit).
  - DMA count minimized (~23/rep): single-shot weights, packed rope
    tables, hoisted v-template constants, and one output DMA per strip
    pair (15-k, k) via a strided DRAM access pattern. Output is bf16.
"""

import sys
import numpy as np

if '/opt/trn_rl_repo' not in sys.path:
    sys.path.insert(0, '/opt/trn_rl_repo')

B, T, C = 2, 2048, 2048
NH, NKV, HD = 32, 8, 64
G = 4              # head groups (TP degree)
QF = NH // G * HD  # 512 q features per core
KF = NKV // G * HD # 128 k/v features per core
NE = 4             # token chunks in projection phase
TE = T // NE       # 256
NCC = C // 128     # 16 contraction chunks
NG = 4             # weight ci-groups (startup overlap)
CG = NCC // NG
NTS = T // 128     # 16 token strips (attention)
VW = 80            # padded v free width for DoubleRow (65 -> 80, 16B align)
VSCALE = 16.0      # host scale on Wv; ones column = VSCALE cancels it

_CACHE = {}


def _rope_perm(nheads):
    """Per-head feature permutation: [d0,d2,...,d62, d1,d3,...,d63]."""
    p = []
    for h in range(nheads):
        base = h * HD
        p.extend(base + d for d in range(0, HD, 2))
        p.extend(base + d for d in range(1, HD, 2))
    return np.array(p, dtype=np.int64)


def _build_program(causal, rep=1):
    assert causal, "v2 kernel is causal-only (harness mask is causal)"
    import concourse.bass as bass
    import concourse.tile as tile
    import concourse.mybir as mybir
    from concourse import bacc
    from concourse.bass import ts

    F32 = mybir.dt.float32
    F32R = mybir.dt.float32r
    BF16 = mybir.dt.bfloat16
    F8 = mybir.dt.float8e4
    Exp = mybir.ActivationFunctionType.Exp
    DR = mybir.MatmulPerfMode.DoubleRow

    nc = bacc.Bacc("TRN2", target_bir_lowering=False, debug=False)

    # pre-tiled inputs: per-partition data is contiguous in DRAM
    xTd = nc.dram_tensor("xTd", [128, NE, NCC, TE], BF16, kind="ExternalInput").ap()
    wqd = nc.dram_tensor("wqd", [128, NCC, QF], BF16, kind="ExternalInput").ap()
    wkd = nc.dram_tensor("wkd", [128, NCC, KF], BF16, kind="ExternalInput").ap()
    wvd = nc.dram_tensor("wvd", [128, NCC, KF], BF16, kind="ExternalInput").ap()
    wod = nc.dram_tensor("wod", [128, QF // 128, C], F32R, kind="ExternalInput").ap()
    csi = nc.dram_tensor("csi", [128, 2 * T + 64], F32, kind="ExternalInput").ap()
    maskd = nc.dram_tensor("maskd", [128, 512], F32R, kind="ExternalInput").ap()
    id128d = nc.dram_tensor("id128d", [128, 128], F32R, kind="ExternalInput").ap()
    seld = nc.dram_tensor("seld", [33, 128], F32R, kind="ExternalInput").ap()
    v8td = nc.dram_tensor("v8td", [128, 2 * 2 * (NTS // 2) * VW], F8,
                          kind="ExternalInput").ap()
    on16d = nc.dram_tensor("on16d", [128, 2, NTS], F32R,
                           kind="ExternalInput").ap()
    zrecd = nc.dram_tensor("zrecd", [33, 512], F32R,
                           kind="ExternalInput").ap()
    part = nc.dram_tensor("part", [T, C], BF16, kind="ExternalOutput").ap()

    with tile.TileContext(nc) as tc:
        from contextlib import ExitStack
        with ExitStack() as ctx:
            persist = ctx.enter_context(tc.tile_pool(name="persist", bufs=1))
            kT_sbp = persist.tile([128, T], F32R)    # roped k^T (kv0 | kv1)
            qp_sb = persist.tile([128, 4, T], F32R)  # packed q^T [64*kv+d, slot, t]
            SLOT = {0: 0, 1: 2, 2: 1, 3: 3}
            # fp8 v pairs: [p, pair-slot, kv, pairidx, VW]; col 64 = VSCALE,
            # cols 65.. = 0
            v8_sb = persist.tile([128, 2, 2, NTS // 2, VW], F8)
            # f32r v for the diagonal strip: [p, kv, strip, 65]
            v1f_sb = persist.tile([128, 2, NTS, HD + 1], F32R)
            maskd_sb = persist.tile([128, 512], F32R)
            nc.sync.dma_start(maskd_sb, maskd)
            id128_sb = persist.tile([128, 128], F32R)
            nc.sync.dma_start(id128_sb, id128d)
            sel_sb = persist.tile([33, 128], F32R)
            nc.sync.dma_start(sel_sb, seld)
            rec_p = persist.tile([33, 512], F32R)
            nc.sync.dma_start(rec_p, zrecd)
            # constant columns of the v tiles: loaded once (compute never
            # writes them)
            nc.sync.dma_start(v1f_sb[:, :, :, HD:HD + 1].squeeze(), on16d)
            nc.sync.dma_start(
                v8_sb.rearrange("p a b c d -> p (a b c d)"), v8td)

            wo_sb = persist.tile([128, QF // 128, C], F32R)
            for _rep in range(rep):
                # ------------- Phase A: QKV projections + RoPE -------------
                with tc.tile_pool(name="stage_a", bufs=1) as stage_a:
                    vT_sb = stage_a.tile([128, T], F32)
                    csi_sb = stage_a.tile([128, 2 * T + 64], F32)
                    c2_sb = csi_sb[:, 0:T]
                    s2_sb = csi_sb[:, T:2 * T]
                    ident = csi_sb[:, 2 * T:2 * T + 64]

                    with tc.tile_pool(name="weights", bufs=1) as wpool, \
                         tc.tile_pool(name="xpool", bufs=2) as xpool, \
                         tc.tile_pool(name="rtmp", bufs=2) as rpool, \
                         tc.tile_pool(name="psT", bufs=2, space="PSUM") as psT, \
                         tc.tile_pool(name="psA", bufs=4, space="PSUM") as psA:
                        # DMA order: weights first (phase-A critical), then
                        # the first x chunk; rope tables + wo follow.
                        wq_sb = wpool.tile([128, NCC, QF], BF16, name="wq_sb")
                        nc.sync.dma_start(wq_sb, wqd)
                        wk_sb = wpool.tile([128, NCC, KF], BF16, name="wk_sb")
                        nc.sync.dma_start(wk_sb, wkd)
                        wv_sb = wpool.tile([128, NCC, KF], BF16, name="wv_sb")
                        nc.sync.dma_start(wv_sb, wvd)

                        for e in range(NE):
                            x_t = xpool.tile([128, NCC, TE], BF16, tag="x")
                            nc.sync.dma_start(x_t, xTd[:, e, :, :])
                            if e == 0:
                                nc.sync.dma_start(csi_sb, csi)
                            elif e == 1:
                                # phase-B inputs: overlap with remaining A
                                nc.sync.dma_start(wo_sb, wod)
                            for f in range(6):  # 0..3 q-tiles, 4 k, 5 v
                                ps = psA.tile([128, TE], F32, tag="psA")
                                for ci in range(NCC):
                                    if f < 4:
                                        w_ap = wq_sb[:, ci, ts(f, 128)]
                                    elif f == 4:
                                        w_ap = wk_sb[:, ci, :]
                                    else:
                                        w_ap = wv_sb[:, ci, :]
                                    nc.tensor.matmul(
                                        ps, w_ap, x_t[:, ci, :],
                                        start=(ci == 0), stop=(ci == NCC - 1))
                                if f == 5:
                                    nc.scalar.activation(
                                        vT_sb[:, ts(e, TE)], ps,
                                        mybir.ActivationFunctionType.Copy)
                                    # transpose this chunk's two strips now
                                    for j in range(4 * e, 4 * e + 4):
                                        for kvh in range(2):
                                            pst = psT.tile([128, HD], F32,
                                                           tag="vtr")
                                            nc.tensor.transpose(
                                                pst,
                                                vT_sb[64 * kvh:64 * kvh + 64,
                                                      ts(j, 128)],
                                                ident[64 * kvh:64 * kvh + 64,
                                                      :])
                                            nc.scalar.activation(
                                                v1f_sb[:, kvh, j, 0:HD], pst,
                                                mybir.ActivationFunctionType
                                                .Copy)
                                            nc.vector.tensor_copy(
                                                v8_sb[:, j % 2, kvh, j // 2,
                                                      0:HD], pst)
                                    continue
                                # RoPE: Act builds the +-32 partition-swapped
                                # copy; DVE muls read PSUM directly and the
                                # final adds scatter to the packed layouts.
                                shifted = rpool.tile([128, TE], F32, tag="sh")
                                for blk in range(4):
                                    o = 32 * blk
                                    so = o + 32 if blk % 2 == 0 else o - 32
                                    nc.scalar.activation(
                                        shifted[o:o + 32, :],
                                        ps[so:so + 32, :],
                                        mybir.ActivationFunctionType.Copy)
                                t1 = rpool.tile([128, TE], F32, tag="t1")
                                nc.vector.tensor_mul(
                                    t1, ps, c2_sb[:, ts(e, TE)])
                                t2 = rpool.tile([128, TE], F32, tag="t2")
                                nc.vector.tensor_mul(
                                    t2, shifted, s2_sb[:, ts(e, TE)])
                                if f == 4:
                                    nc.vector.tensor_add(
                                        kT_sbp[:, ts(e, TE)], t1, t2)
                                else:
                                    for s in range(2):
                                        h = 2 * f + s
                                        kv, slot = h // 4, SLOT[h % 4]
                                        nc.vector.tensor_add(
                                            qp_sb[64 * kv:64 * kv + 64, slot,
                                                  ts(e, TE)],
                                            t1[64 * s:64 * s + 64, :],
                                            t2[64 * s:64 * s + 64, :])

                # ------------- Phase B: attention + o_proj -------------
                with tc.tile_pool(name="pp", bufs=3) as ppool, \
                     tc.tile_pool(name="norm", bufs=4) as npool, \
                     tc.tile_pool(name="atts", bufs=2) as apool, \
                     tc.tile_pool(name="outs", bufs=2) as opool, \
                     tc.tile_pool(name="psS", bufs=2, space="PSUM") as psS, \
                     tc.tile_pool(name="psO", bufs=2, space="PSUM") as psO:

                    prev = None  # deferred o_proj: (att_s, cstrip, o2, sl)
                    part16 = part.rearrange("(a b) c -> a b c", a=NTS)

                    def emit_oproj_half(att_prev, cs_prev, o2t, sl, ecp):
                        # pc rides the po tag rings: po is double-buffered and
                        # o_proj PSUM reuses the same 4 banks.
                        for sub in range(2):
                            ec = 2 * ecp + sub
                            pc = psO.tile([128, 512], F32, tag=f"po{sub}",
                                          name=f"pc{sub}")
                            for ff in range(4):
                                nc.tensor.matmul(
                                    pc, att_prev[:, ff, :],
                                    wo_sb[:, ff, ts(ec, 512)],
                                    start=(ff == 0), stop=(ff == 3))
                            nc.vector.tensor_copy(
                                o2t[:, sl, ts(ec, 512)], pc)
                        if ecp == 1 and cs_prev < NTS // 2:
                            # pair (NTS-1-k, k) complete -> one DMA for both
                            k = cs_prev
                            dst = part16[k:NTS - k:NTS - 1 - 2 * k, :, :]
                            nc.sync.dma_start(
                                dst.rearrange("a b c -> b a c"), o2t)


                    # interleave big/small strips so the small strips'
                    # normalize + o_proj latency hides under big strips'
                    # attention work
                    order = []
                    for i in range(NTS // 2):
                        order.extend([NTS - 1 - i, i])
                    for cstrip in order:
                        nstr = cstrip + 1
                        po = {}
                        for kv in range(2):
                            po[kv] = psO.tile([128, 512], F32, tag=f"po{kv}",
                                              name=f"po{kv}")[0:VW, :]
                            npairs = (nstr + 1) // 2
                            for pr in range(npairs):
                                j0, j1 = 2 * pr, 2 * pr + 1
                                pss = psS.tile([128, 2, 512], F32, tag="pss")
                                for sl, j in ((0, j0), (1, j1)):
                                    if j > cstrip:
                                        continue
                                    diag = (j == cstrip)
                                    if diag:
                                        # seed the slot with the causal mask
                                        nc.tensor.matmul(
                                            pss[:, sl, :], id128_sb,
                                            maskd_sb, start=True, stop=False,
                                            skip_group_check=True)
                                    nc.tensor.matmul(
                                        pss[:, sl, :],
                                        kT_sbp[64 * kv:64 * kv + 64,
                                               ts(j, 128)],
                                        qp_sb[64 * kv:64 * kv + 64, :,
                                              ts(cstrip, 128)],
                                        start=not diag, stop=True,
                                        skip_group_check=True)
                                dslot = (0 if j0 == cstrip else
                                         (1 if j1 == cstrip else None))
                                if dslot is not None:
                                    # off-diag slot (if any) -> fp8 + DR PV
                                    if dslot == 1:
                                        p8 = ppool.tile([128, 2, 512], F8,
                                                        tag="p8")
                                        nc.scalar.activation(
                                            p8[:, 0, :], pss[:, 0, :], Exp,
                                            scale=0.125)
                                        nc.any.memset(p8[:, 1, :], 0.0)
                                        nc.tensor.matmul(
                                            po[kv], v8_sb[:, :, kv, pr, :],
                                            p8, start=(pr == 0), stop=False,
                                            perf_mode=DR,
                                            skip_group_check=True)
                                    # diagonal strip: f32r p and f32r PV
                                    p32 = ppool.tile([128, 512], F32R,
                                                     tag="p32")
                                    nc.scalar.activation(
                                        p32, pss[:, dslot, :], Exp,
                                        scale=0.125)
                                    nc.tensor.matmul(
                                        po[kv][0:HD + 1, :],
                                        v1f_sb[:, kv, cstrip, :],
                                        p32,
                                        start=(pr == 0 and dslot == 0),
                                        stop=True, skip_group_check=True)
                                else:
                                    p8 = ppool.tile([128, 2, 512], F8,
                                                    tag="p8")
                                    nc.scalar.activation(
                                        p8, pss, Exp, scale=0.125)
                                    nc.tensor.matmul(
                                        po[kv], v8_sb[:, :, kv, pr, :],
                                        p8, start=(pr == 0), stop=False,
                                        perf_mode=DR, skip_group_check=True)
                            if prev is not None:
                                emit_oproj_half(prev[0], prev[1], prev[2], prev[3], kv)

                        # normalize both kv heads: one fused broadcast
                        # (kv1 sums at partition 32: bases must be quadrant-
                        # aligned; rec rows 1..31 are zeroed once at start)
                        nc.vector.tensor_copy(rec_p[0:1, :],
                                              po[0][HD:HD + 1, :])
                        nc.vector.tensor_copy(rec_p[32:33, :],
                                              po[1][HD:HD + 1, :])
                        recb_t = psS.tile([128, 2, 512], F32, tag="pss")
                        nc.tensor.matmul(recb_t[:, 0, :], sel_sb, rec_p,
                                         start=True, stop=True)
                        recb = npool.tile([128, 512], F32, tag="recbs")
                        nc.vector.reciprocal(recb, recb_t[:, 0, :])
                        att_s = apool.tile([128, 4, 128], F32R, tag="att")
                        for kv in range(2):
                            for half in range(2):
                                nc.vector.tensor_mul(
                                    att_s[64 * half:64 * half + 64,
                                          2 * kv:2 * kv + 2, :],
                                    po[kv][0:64, ts(half, 256)].rearrange(
                                        "p (a b) -> p a b", a=2),
                                    recb[64 * kv:64 * kv + 64,
                                         ts(half, 256)].rearrange(
                                        "p (a b) -> p a b", a=2))

                        if cstrip >= NTS // 2:
                            o2t = opool.tile([128, 2, C], BF16, tag="o2",
                                             name="o2t")
                            cur_o2 = o2t
                        sl = 1 if cstrip >= NTS // 2 else 0
                        prev = (att_s, cstrip, cur_o2, sl)
                    emit_oproj_half(prev[0], prev[1], prev[2], prev[3], 0)
                    emit_oproj_half(prev[0], prev[1], prev[2], prev[3], 1)

    nc.finalize()
    return nc


def _host_inputs(x, freqs_cos, freqs_sin, attention_mask, Wq, Wk, Wv, Wo,
                 causal):
    """Build the 8 per-core input maps (pre-tiled for contiguous DMA)."""
    import ml_dtypes
    bf16 = ml_dtypes.bfloat16
    fc = np.asarray(freqs_cos, np.float32)
    fs = np.asarray(freqs_sin, np.float32)
    mask = np.asarray(attention_mask, np.float32)

    # rope factor tables [128, T]; pattern repeats every 64 partitions
    m_idx = np.tile(np.arange(32), 4)                     # p % 32
    c2 = np.ascontiguousarray(fc.T[m_idx])                # [128, T]
    sgn = np.where((np.arange(128) % 64) < 32, -1.0, 1.0).astype(np.float32)
    s2 = np.ascontiguousarray(fs.T[m_idx] * sgn[:, None])
    ident64 = np.ascontiguousarray(
        np.tile(np.eye(64, dtype=np.float32), (2, 1)))
    csi = np.ascontiguousarray(
        np.concatenate([c2, s2, ident64], axis=1)).astype(np.float32)

    # diagonal-block mask, transposed + pre-scaled by 8 (kernel applies *0.125)
    md = (mask[0:128, 0:128].T * 8.0).astype(np.float32)
    maskd = np.ascontiguousarray(
        np.broadcast_to(md[:, None, :], (128, 4, 128))
        .reshape(128, 512)).astype(np.float32)
    id128 = np.eye(128, dtype=np.float32)

    qperm = _rope_perm(NH // G)
    kperm = _rope_perm(NKV // G)

    # recb selector: kv0 sums live at partition 0, kv1 sums at partition 32
    seld = np.zeros((33, 128), np.float32)
    seld[0, 0:64] = 1.0
    seld[32, 64:128] = 1.0

    # v8 template: zeros with the ones-column = VSCALE at col 64
    f8 = ml_dtypes.float8_e4m3
    v8t = np.zeros((128, 2, 2, NTS // 2, VW), np.float32)
    v8t[:, :, :, :, HD] = VSCALE
    v8t = np.ascontiguousarray(v8t.reshape(128, -1)).astype(f8)
    on16 = np.full((128, 2, NTS), VSCALE, np.float32)

    def tile_w(wT, F):
        # [C, F] -> [128, NCC, F], contiguous per partition
        return np.ascontiguousarray(
            np.ascontiguousarray(wT).reshape(NCC, 128, F)
            .transpose(1, 0, 2)).astype(bf16)

    in_maps = []
    for c in range(8):
        b, g = c // 4, c % 4
        xT = np.asarray(x, np.float32)[b].T  # [C, T]
        xtile = np.ascontiguousarray(
            xT.reshape(NCC, 128, NE, TE).transpose(1, 2, 0, 3)).astype(bf16)
        wqT = np.asarray(Wq, np.float32)[g * QF:(g + 1) * QF][qperm].T
        wkT = np.asarray(Wk, np.float32)[g * KF:(g + 1) * KF][kperm].T
        wvT = np.asarray(Wv, np.float32)[g * KF:(g + 1) * KF].T * VSCALE
        woT = np.asarray(Wo, np.float32)[:, g * QF:(g + 1) * QF].T  # [QF, C]
        m = {
            "xTd": xtile,
            "wqd": tile_w(wqT, QF),
            "wkd": tile_w(wkT, KF),
            "wvd": tile_w(wvT, KF),
            "wod": np.ascontiguousarray(
                woT.reshape(QF // 128, 128, C).transpose(1, 0, 2)),
            "csi": csi,
            "maskd": maskd,
            "id128d": id128,
            "seld": seld,
            "v8td": v8t,
            "on16d": on16,
            "zrecd": np.zeros((33, 512), np.float32),
        }
        in_maps.append(m)
    return in_maps


def _detect_causal(mask):
    mask = np.asarray(mask)
    neg = mask.min()
    if neg >= -1e7:
        return False
    tril = np.tril(np.ones((T, T), dtype=bool))
    expect = np.where(tril, np.float32(0.0), np.float32(neg))
    return bool(np.array_equal(mask, expect))


def run(inputs, trace=False):
    from concourse import bass_utils

    causal = _detect_causal(inputs["attention_mask"])
    key = ("prog", causal)
    if key not in _CACHE:
        _CACHE[key] = _build_program(causal)
    nc = _CACHE[key]

    in_maps = _host_inputs(causal=causal, **inputs)
    res = bass_utils.run_bass_kernel_spmd(
        nc, in_maps, core_ids=list(range(8)), trace=trace)

    out = np.empty((B, T, C), np.float32)
    for b in range(B):
        acc = res.results[4 * b]["part"].astype(np.float32)
        for g in range(1, 4):
            acc = acc + res.results[4 * b + g]["part"].astype(np.float32)
        out[b] = acc
    return out, res


def kernel(**inputs):
    out, _ = run(inputs, trace=False)
    return out


# revision 49
# speedup vs baseline: 2.0294x; 1.0335x over previous
"""Llama SDPA attention (GQA + RoPE + causal) on 8 Trainium2 NeuronCores.

Sharding: DP-2 over batch x TP-4 over heads. Core c = 4*b + g handles
batch b and head group g (8 q heads, 2 kv heads). Each core computes its
partial o_proj output [T, C] (Wo split along the input-feature dim); the
partials are summed on the host (the gather/unshard step).

v2 kernel notes (on top of the v1 design):
  - Phase A in bf16 (x moving + W stationary, NE=4 chunks of 512 tokens):
    same PE rate as fp32r, half the DMA bytes, half the RoPE/copy
    instruction count. Wv is host-scaled x16 so v is fp8-friendly; the
    softmax ones-column is 16 so normalization cancels the scale exactly.
  - RoPE: the straight mul reads the projection PSUM directly (engine ops
    may read PSUM/SBUF operands at shifted partition bases); only the
    +-32 partition-swapped operand needs Act copies. The rotated q is
    scatter-added straight into the packed q layout.
  - Attention processes key strips in PAIRS: QK writes a [128,2,512] PSUM
    pair tile, one Act exp converts both strips to fp8e4, and one
    DoubleRow fp8 matmul (2 strips = 256-deep contraction) accumulates PV.
    The diagonal strip stays fp32r end-to-end (exp -> f32r p, f32r PV via
    a separate f32r v copy) - early-token rows see no fp8 error, and
    off-diagonal fp8 rounding averages out over many keys (~6e-3 total).
    The causal mask is seeded into the diag PSUM by an identity matmul so
    no DVE hop sits on the Act critical path. Strips run in interleaved
    big/small order so small strips' normalize/o_proj latency hides under
    big strips' attention.
  - o_proj stays fp32r (fp8 would need both-operand residual compensation
    to pass the accuracy gate - not worth it), is software-pipelined by
    one strip, and its PSUM rides the po accumulator rings (8-bank fit).
  - DMA count minimized (~23/rep): single-shot weights, packed rope
    tables, hoisted v-template constants, and one output DMA per strip
    pair (15-k, k) via a strided DRAM access pattern. Output is bf16.
"""

import sys
import numpy as np

if '/opt/trn_rl_repo' not in sys.path:
    sys.path.insert(0, '/opt/trn_rl_repo')

B, T, C = 2, 2048, 2048
NH, NKV, HD = 32, 8, 64
G = 4              # head groups (TP degree)
QF = NH // G * HD  # 512 q features per core
KF = NKV // G * HD # 128 k/v features per core
NE = 4             # token chunks in projection phase
TE = T // NE       # 256
NCC = C // 128     # 16 contraction chunks
NG = 4             # weight ci-groups (startup overlap)
CG = NCC // NG
NTS = T // 128     # 16 token strips (attention)
VW = 80            # padded v free width for DoubleRow (65 -> 80, 16B align)
VSCALE = 16.0      # host scale on Wv; ones column = VSCALE cancels it

_CACHE = {}


def _rope_perm(nheads):
    """Per-head feature permutation: [d0,d2,...,d62, d1,d3,...,d63]."""
    p = []
    for h in range(nheads):
        base = h * HD
        p.extend(base + d for d in range(0, HD, 2))
        p.extend(base + d for d in range(1, HD, 2))
    return np.array(p, dtype=np.int64)


def _build_program(causal, rep=1):
    assert causal, "v2 kernel is causal-only (harness mask is causal)"
    import concourse.bass as bass
    import concourse.tile as tile
    import concourse.mybir as mybir
    from concourse import bacc
    from concourse.bass import ts

    F32 = mybir.dt.float32
    F32R = mybir.dt.float32r
    BF16 = mybir.dt.bfloat16
    F8 = mybir.dt.float8e4
    Exp = mybir.ActivationFunctionType.Exp
    DR = mybir.MatmulPerfMode.DoubleRow

    nc = bacc.Bacc("TRN2", target_bir_lowering=False, debug=False)

    # pre-tiled inputs: per-partition data is contiguous in DRAM
    xTd = nc.dram_tensor("xTd", [128, NE, NCC, TE], BF16, kind="ExternalInput").ap()
    wqd = nc.dram_tensor("wqd", [128, NCC, QF], BF16, kind="ExternalInput").ap()
    wkd = nc.dram_tensor("wkd", [128, NCC, KF], BF16, kind="ExternalInput").ap()
    wvd = nc.dram_tensor("wvd", [128, NCC, KF], BF16, kind="ExternalInput").ap()
    wod = nc.dram_tensor("wod", [128, QF // 128, C], F32R, kind="ExternalInput").ap()
    csi = nc.dram_tensor("csi", [128, 2 * T + 64], F32, kind="ExternalInput").ap()
    maskd = nc.dram_tensor("maskd", [128, 512], F32R, kind="ExternalInput").ap()
    id128d = nc.dram_tensor("id128d", [128, 128], F32R, kind="ExternalInput").ap()
    seld = nc.dram_tensor("seld", [33, 128], F32R, kind="ExternalInput").ap()
    v8td = nc.dram_tensor("v8td", [128, 2 * 2 * (NTS // 2) * VW], F8,
                          kind="ExternalInput").ap()
    on16d = nc.dram_tensor("on16d", [128, 2, NTS], F32R,
                           kind="ExternalInput").ap()
    zrecd = nc.dram_tensor("zrecd", [33, 512], F32R,
                           kind="ExternalInput").ap()
    part = nc.dram_tensor("part", [T, C], BF16, kind="ExternalOutput").ap()

    with tile.TileContext(nc) as tc:
        from contextlib import ExitStack
        with ExitStack() as ctx:
            persist = ctx.enter_context(tc.tile_pool(name="persist", bufs=1))
            kT_sbp = persist.tile([128, T], F32R)    # roped k^T (kv0 | kv1)
            qp_sb = persist.tile([128, 4, T], F32R)  # packed q^T [64*kv+d, slot, t]
            SLOT = {0: 0, 1: 2, 2: 1, 3: 3}
            # fp8 v pairs: [p, pair-slot, kv, pairidx, VW]; col 64 = VSCALE,
            # cols 65.. = 0
            v8_sb = persist.tile([128, 2, 2, NTS // 2, VW], F8)
            # f32r v for the diagonal strip: [p, kv, strip, 65]
            v1f_sb = persist.tile([128, 2, NTS, HD + 1], F32R)
            maskd_sb = persist.tile([128, 512], F32R)
            nc.sync.dma_start(maskd_sb, maskd)
            id128_sb = persist.tile([128, 128], F32R)
            nc.sync.dma_start(id128_sb, id128d)
            sel_sb = persist.tile([33, 128], F32R)
            nc.sync.dma_start(sel_sb, seld)
            rec_p = persist.tile([33, 512], F32R)
            nc.sync.dma_start(rec_p, zrecd)
            # constant columns of the v tiles: loaded once (compute never
            # writes them)
            nc.sync.dma_start(v1f_sb[:, :, :, HD:HD + 1].squeeze(), on16d)
            nc.sync.dma_start(
                v8_sb.rearrange("p a b c d -> p (a b c d)"), v8td)

            wo_sb = persist.tile([128, QF // 128, C], F32R)
            for _rep in range(rep):
                # ------------- Phase A: QKV projections + RoPE -------------
                with tc.tile_pool(name="stage_a", bufs=1) as stage_a:
                    vT_sb = stage_a.tile([128, T], F32)
                    csi_sb = stage_a.tile([128, 2 * T + 64], F32)
                    c2_sb = csi_sb[:, 0:T]
                    s2_sb = csi_sb[:, T:2 * T]
                    ident = csi_sb[:, 2 * T:2 * T + 64]

                    with tc.tile_pool(name="weights", bufs=1) as wpool, \
                         tc.tile_pool(name="xpool", bufs=2) as xpool, \
                         tc.tile_pool(name="rtmp", bufs=2) as rpool, \
                         tc.tile_pool(name="psT", bufs=2, space="PSUM") as psT, \
                         tc.tile_pool(name="psA", bufs=4, space="PSUM") as psA:
                        # DMA order: weights first (phase-A critical), then
                        # the first x chunk; rope tables + wo follow.
                        wq_sb = wpool.tile([128, NCC, QF], BF16, name="wq_sb")
                        nc.sync.dma_start(wq_sb, wqd)
                        wk_sb = wpool.tile([128, NCC, KF], BF16, name="wk_sb")
                        nc.sync.dma_start(wk_sb, wkd)
                        wv_sb = wpool.tile([128, NCC, KF], BF16, name="wv_sb")
                        nc.sync.dma_start(wv_sb, wvd)

                        for e in range(NE):
                            x_t = xpool.tile([128, NCC, TE], BF16, tag="x")
                            nc.sync.dma_start(x_t, xTd[:, e, :, :])
                            if e == 0:
                                nc.sync.dma_start(csi_sb, csi)
                            elif e == 1:
                                # phase-B inputs: overlap with remaining A
                                nc.sync.dma_start(wo_sb, wod)
                            for f in range(6):  # 0..3 q-tiles, 4 k, 5 v
                                ps = psA.tile([128, TE], F32, tag="psA")
                                for ci in range(NCC):
                                    if f < 4:
                                        w_ap = wq_sb[:, ci, ts(f, 128)]
                                    elif f == 4:
                                        w_ap = wk_sb[:, ci, :]
                                    else:
                                        w_ap = wv_sb[:, ci, :]
                                    nc.tensor.matmul(
                                        ps, w_ap, x_t[:, ci, :],
                                        start=(ci == 0), stop=(ci == NCC - 1))
                                if f == 5:
                                    nc.scalar.activation(
                                        vT_sb[:, ts(e, TE)], ps,
                                        mybir.ActivationFunctionType.Copy)
                                    # transpose this chunk's two strips now
                                    for j in range(4 * e, 4 * e + 4):
                                        for kvh in range(2):
                                            pst = psT.tile([128, HD], F32,
                                                           tag="vtr")
                                            nc.tensor.transpose(
                                                pst,
                                                vT_sb[64 * kvh:64 * kvh + 64,
                                                      ts(j, 128)],
                                                ident[64 * kvh:64 * kvh + 64,
                                                      :])
                                            nc.scalar.activation(
                                                v1f_sb[:, kvh, j, 0:HD], pst,
                                                mybir.ActivationFunctionType
                                                .Copy)
                                            nc.vector.tensor_copy(
                                                v8_sb[:, j % 2, kvh, j // 2,
                                                      0:HD], pst)
                                    continue
                                # RoPE: Act builds the +-32 partition-swapped
                                # copy; DVE muls read PSUM directly and the
                                # final adds scatter to the packed layouts.
                                shifted = rpool.tile([128, TE], F32, tag="sh")
                                for blk in range(4):
                                    o = 32 * blk
                                    so = o + 32 if blk % 2 == 0 else o - 32
                                    nc.scalar.activation(
                                        shifted[o:o + 32, :],
                                        ps[so:so + 32, :],
                                        mybir.ActivationFunctionType.Copy)
                                t1 = rpool.tile([128, TE], F32, tag="t1")
                                nc.vector.tensor_mul(
                                    t1, ps, c2_sb[:, ts(e, TE)])
                                t2 = rpool.tile([128, TE], F32, tag="t2")
                                nc.vector.tensor_mul(
                                    t2, shifted, s2_sb[:, ts(e, TE)])
                                if f == 4:
                                    nc.vector.tensor_add(
                                        kT_sbp[:, ts(e, TE)], t1, t2)
                                else:
                                    for s in range(2):
                                        h = 2 * f + s
                                        kv, slot = h // 4, SLOT[h % 4]
                                        nc.vector.tensor_add(
                                            qp_sb[64 * kv:64 * kv + 64, slot,
                                                  ts(e, TE)],
                                            t1[64 * s:64 * s + 64, :],
                                            t2[64 * s:64 * s + 64, :])

                # ------------- Phase B: attention + o_proj -------------
                with tc.tile_pool(name="pp", bufs=3) as ppool, \
                     tc.tile_pool(name="norm", bufs=4) as npool, \
                     tc.tile_pool(name="atts", bufs=2) as apool, \
                     tc.tile_pool(name="outs", bufs=2) as opool, \
                     tc.tile_pool(name="psS", bufs=2, space="PSUM") as psS, \
                     tc.tile_pool(name="psO", bufs=2, space="PSUM") as psO:

                    prev = None  # deferred o_proj: (att_s, cstrip, o2, sl)
                    part16 = part.rearrange("(a b) c -> a b c", a=NTS)

                    def emit_oproj_half(att_prev, cs_prev, o2t, sl, ecp):
                        # pc rides the po tag rings: po is double-buffered and
                        # o_proj PSUM reuses the same 4 banks.
                        for sub in range(2):
                            ec = 2 * ecp + sub
                            pc = psO.tile([128, 512], F32, tag=f"po{sub}",
                                          name=f"pc{sub}")
                            for ff in range(4):
                                nc.tensor.matmul(
                                    pc, att_prev[:, ff, :],
                                    wo_sb[:, ff, ts(ec, 512)],
                                    start=(ff == 0), stop=(ff == 3))
                            nc.vector.tensor_copy(
                                o2t[:, sl, ts(ec, 512)], pc)
                        if ecp == 1 and cs_prev < NTS // 2:
                            # pair (NTS-1-k, k) complete -> one DMA for both
                            k = cs_prev
                            dst = part16[k:NTS - k:NTS - 1 - 2 * k, :, :]
                            nc.sync.dma_start(
                                dst.rearrange("a b c -> b a c"), o2t)


                    # interleave big/small strips so the small strips'
                    # normalize + o_proj latency hides under big strips'
                    # attention work
                    order = []
                    for i in range(NTS // 2):
                        order.extend([NTS - 1 - i, i])
                    for cstrip in order:
                        nstr = cstrip + 1
                        po = {}
                        for kv in range(2):
                            po[kv] = psO.tile([128, 512], F32, tag=f"po{kv}",
                                              name=f"po{kv}")[0:VW, :]
                            npairs = (nstr + 1) // 2
                            for pr in range(npairs):
                                j0, j1 = 2 * pr, 2 * pr + 1
                                pss = psS.tile([128, 2, 512], F32, tag="pss")
                                for sl, j in ((0, j0), (1, j1)):
                                    if j > cstrip:
                                        continue
                                    diag = (j == cstrip)
                                    if diag:
                                        # seed the slot with the causal mask
                                        nc.tensor.matmul(
                                            pss[:, sl, :], id128_sb,
                                            maskd_sb, start=True, stop=False,
                                            skip_group_check=True)
                                    nc.tensor.matmul(
                                        pss[:, sl, :],
                                        kT_sbp[64 * kv:64 * kv + 64,
                                               ts(j, 128)],
                                        qp_sb[64 * kv:64 * kv + 64, :,
                                              ts(cstrip, 128)],
                                        start=not diag, stop=True,
                                        skip_group_check=True)
                                dslot = (0 if j0 == cstrip else
                                         (1 if j1 == cstrip else None))
                                if dslot is not None:
                                    # off-diag slot (if any) -> fp8 + DR PV
                                    if dslot == 1:
                                        p8 = ppool.tile([128, 2, 512], F8,
                                                        tag="p8")
                                        nc.scalar.activation(
                                            p8[:, 0, :], pss[:, 0, :], Exp,
                                            scale=0.125)
                                        nc.any.memset(p8[:, 1, :], 0.0)
                                        nc.tensor.matmul(
                                            po[kv], v8_sb[:, :, kv, pr, :],
                                            p8, start=(pr == 0), stop=False,
                                            perf_mode=DR,
                                            skip_group_check=True)
                                    # diagonal strip: f32r p and f32r PV
                                    p32 = ppool.tile([128, 512], F32R,
                                                     tag="p32")
                                    nc.scalar.activation(
                                        p32, pss[:, dslot, :], Exp,
                                        scale=0.125)
                                    nc.tensor.matmul(
                                        po[kv][0:HD + 1, :],
                                        v1f_sb[:, kv, cstrip, :],
                                        p32,
                                        start=(pr == 0 and dslot == 0),
                                        stop=True, skip_group_check=True)
                                else:
                                    p8 = ppool.tile([128, 2, 512], F8,
                                                    tag="p8")
                                    nc.scalar.activation(
                                        p8, pss, Exp, scale=0.125)
                                    nc.tensor.matmul(
                                        po[kv], v8_sb[:, :, kv, pr, :],
                                        p8, start=(pr == 0), stop=False,
                                        perf_mode=DR, skip_group_check=True)
                            if prev is not None:
                                emit_oproj_half(prev[0], prev[1], prev[2], prev[3], kv)

                        # normalize both kv heads: one fused broadcast
                        # (kv1 sums at partition 32: bases must be quadrant-
                        # aligned; rec rows 1..31 are zeroed once at start)
                        nc.vector.tensor_copy(rec_p[0:1, :],
                                              po[0][HD:HD + 1, :])
                        nc.vector.tensor_copy(rec_p[32:33, :],
                                              po[1][HD:HD + 1, :])
                        recb_t = psS.tile([128, 2, 512], F32, tag="pss")
                        nc.tensor.matmul(recb_t[:, 0, :], sel_sb, rec_p,
                                         start=True, stop=True)
                        recb = npool.tile([128, 512], F32, tag="recbs")
                        nc.vector.reciprocal(recb, recb_t[:, 0, :])
                        att_s = apool.tile([128, 4, 128], F32R, tag="att")
                        for kv in range(2):
                            for half in range(2):
                                nc.vector.tensor_mul(
                                    att_s[64 * half:64 * half + 64,
                                          2 * kv:2 * kv + 2, :],
                                    po[kv][0:64, ts(half, 256)].rearrange(
                                        "p (a b) -> p a b", a=2),
                                    recb[64 * kv:64 * kv + 64,
                                         ts(half, 256)].rearrange(
                                        "p (a b) -> p a b", a=2))

                        if cstrip >= NTS // 2:
                            o2t = opool.tile([128, 2, C], BF16, tag="o2",
                                             name="o2t")
                            cur_o2 = o2t
                        sl = 1 if cstrip >= NTS // 2 else 0
                        prev = (att_s, cstrip, cur_o2, sl)
                    emit_oproj_half(prev[0], prev[1], prev[2], prev[3], 0)
                    emit_oproj_half(prev[0], prev[1], prev[2], prev[3], 1)

    nc.finalize()
    return nc


def _host_inputs(x, freqs_cos, freqs_sin, attention_mask, Wq, Wk, Wv, Wo,
                 causal):
    """Build the 8 per-core input maps (pre-tiled for contiguous DMA)."""
    import ml_dtypes
    bf16 = ml_dtypes.bfloat16
    fc = np.asarray(freqs_cos, np.float32)
    fs = np.asarray(freqs_sin, np.float32)
    mask = np.asarray(attention_mask, np.float32)

    # rope factor tables [128, T]; pattern repeats every 64 partitions
    m_idx = np.tile(np.arange(32), 4)                     # p % 32
    c2 = np.ascontiguousarray(fc.T[m_idx])                # [128, T]
    sgn = np.where((np.arange(128) % 64) < 32, -1.0, 1.0).astype(np.float32)
    s2 = np.ascontiguousarray(fs.T[m_idx] * sgn[:, None])
    ident64 = np.ascontiguousarray(
        np.tile(np.eye(64, dtype=np.float32), (2, 1)))
    csi = np.ascontiguousarray(
        np.concatenate([c2, s2, ident64], axis=1)).astype(np.float32)

    # diagonal-block mask, transposed + pre-scaled by 8 (kernel applies *0.125)
    md = (mask[0:128, 0:128].T * 8.0).astype(np.float32)
    maskd = np.ascontiguousarray(
        np.broadcast_to(md[:, None, :], (128, 4, 128))
        .reshape(128, 512)).astype(np.float32)
    id128 = np.eye(128, dtype=np.float32)

    qperm = _rope_perm(NH // G)
    kperm = _rope_perm(NKV // G)

    # recb selector: kv0 sums live at partition 0, kv1 sums at partition 32
    seld = np.zeros((33, 128), np.float32)
    seld[0, 0:64] = 1.0
    seld[32, 64:128] = 1.0

    # v8 template: zeros with the ones-column = VSCALE at col 64
    f8 = ml_dtypes.float8_e4m3
    v8t = np.zeros((128, 2, 2, NTS // 2, VW), np.float32)
    v8t[:, :, :, :, HD] = VSCALE
    v8t = np.ascontiguousarray(v8t.reshape(128, -1)).astype(f8)
    on16 = np.full((128, 2, NTS), VSCALE, np.float32)

    def tile_w(wT, F):
        # [C, F] -> [128, NCC, F], contiguous per partition
        return np.ascontiguousarray(
            np.ascontiguousarray(wT).reshape(NCC, 128, F)
            .transpose(1, 0, 2)).astype(bf16)

    in_maps = []
    for c in range(8):
        b, g = c // 4, c % 4
        xT = np.asarray(x, np.float32)[b].T  # [C, T]
        xtile = np.ascontiguousarray(
            xT.reshape(NCC, 128, NE, TE).transpose(1, 2, 0, 3)).astype(bf16)
        wqT = np.asarray(Wq, np.float32)[g * QF:(g + 1) * QF][qperm].T
        wkT = np.asarray(Wk, np.float32)[g * KF:(g + 1) * KF][kperm].T
        wvT = np.asarray(Wv, np.float32)[g * KF:(g + 1) * KF].T * VSCALE
        woT = np.asarray(Wo, np.float32)[:, g * QF:(g + 1) * QF].T  # [QF, C]
        m = {
            "xTd": xtile,
            "wqd": tile_w(wqT, QF),
            "wkd": tile_w(wkT, KF),
            "wvd": tile_w(wvT, KF),
            "wod": np.ascontiguousarray(
                woT.reshape(QF // 128, 128, C).transpose(1, 0, 2)),
            "csi": csi,
            "maskd": maskd,
            "id128d": id128,
            "seld": seld,
            "v8td": v8t,
            "on16d": on16,
            "zrecd": np.zeros((33, 512), np.float32),
        }
        in_maps.append(m)
    return in_maps


def _detect_causal(mask):
    mask = np.asarray(mask)
    neg = mask.min()
    if neg >= -1e7:
        return False
    tril = np.tril(np.ones((T, T), dtype=bool))
    expect = np.where(tril, np.float32(0.0), np.float32(neg))
    return bool(np.array_equal(mask, expect))


def run(inputs, trace=False):
    from concourse import bass_utils

    causal = _detect_causal(inputs["attention_mask"])
    key = ("prog", causal)
    if key not in _CACHE:
        _CACHE[key] = _build_program(causal)
    nc = _CACHE[key]

    in_maps = _host_inputs(causal=causal, **inputs)
    res = bass_utils.run_bass_kernel_spmd(
        nc, in_maps, core_ids=list(range(8)), trace=trace)

    out = np.empty((B, T, C), np.float32)
    for b in range(B):
        acc = res.results[4 * b]["part"].astype(np.float32)
        for g in range(1, 4):
            acc = acc + res.results[4 * b + g]["part"].astype(np.float32)
        out[b] = acc
    return out, res


def kernel(**inputs):
    out, _ = run(inputs, trace=False)
    return out
